# revision 1
# baseline (speedup 1.0000x reference)
"""ArcFace head on 8 TRN2 NeuronCores — transposed class-parallel layout.

Core c owns classes [c*12500, (c+1)*12500). The cos matmul runs with classes
on the PSUM partition axis: out[c_local, b] = S * (F_hat_b . W_c) * inv|W_c|,
so the post-matmul scale is a per-partition scalar that either the Scalar or
Vector engine can apply (splittable), and the class-norm reduction becomes
98 one-column matmuls with the summed-squares tile as the stationary operand.

Features are pre-normalized on device (S/|f| folded into the moving operand).
The ArcFace margin values for every row are computed on device in a
replicated-row layout (no per-row gather needed) and written to a small vout
tensor; the host applies them to the target columns by fancy indexing
(indexing only, no host FLOPs).

Everything on device runs in fp16 (full PE rate, 10-bit mantissa), with all
reductions accumulating in fp32 PSUM.
"""

import math
import os

import numpy as np

B = 512
D = 512
C = 100000
NCORES = 8
CS = C // NCORES          # 12500 classes per core

M_MARGIN = 0.5
S_SCALE = 64.0
TH = math.cos(math.pi - M_MARGIN)
MM_ = math.sin(math.pi - M_MARGIN) * M_MARGIN

P = 128
NK = D // P               # 4 contraction chunks
NCH = (CS + P - 1) // P   # 98 class chunks (97*128 + 84)
PW = 1024                 # classes per square/partial piece (8 chunks)
NPC = (CS + PW - 1) // PW  # 13 pieces, last = 212 wide

_CACHE = {}


def _pieces():
    out = []
    for p in range(NPC):
        lo = p * PW
        hi = min(CS, lo + PW)
        out.append((lo, hi))
    return out


def _build_nc(opts=None):
    opts = opts or {}
    import concourse.tile as tile
    from concourse import bacc, mybir

    dt = mybir.dt
    Alu = mybir.AluOpType
    Act = mybir.ActivationFunctionType

    nc = bacc.Bacc("TRN2", target_bir_lowering=False, debug=False,
                   enable_asserts=False, num_devices=NCORES)

    wt = nc.dram_tensor("wt", [D, CS], dt.bfloat16, kind="ExternalInput").ap()
    featT = nc.dram_tensor("featT", [D, B], dt.bfloat16,
                           kind="ExternalInput").ap()
    wgT = nc.dram_tensor("wgT", [D, B], dt.bfloat16, kind="ExternalInput").ap()
    out = nc.dram_tensor("out", [CS, B], dt.bfloat16, kind="ExternalOutput").ap()
    vout = nc.dram_tensor("vout", [1, B], dt.float32,
                          kind="ExternalOutput").ap()

    dve_share = opts.get("dve_share", 5)  # posts per 32 chunks on DVE

    with tile.TileContext(nc) as tc:
        with (
            tc.tile_pool(name="const", bufs=1) as constp,
            tc.tile_pool(name="fm", bufs=2) as fmp,      # feature/margin scratch
            tc.tile_pool(name="sqs", bufs=2) as sqp,     # square scratch
            tc.tile_pool(name="outp", bufs=8) as outp,
            tc.tile_pool(name="ps_o", bufs=opts.get("ps_o", 5),
                         space="PSUM") as ps_o,
            tc.tile_pool(name="ps_n", bufs=1, space="PSUM") as ps_n,
            tc.tile_pool(name="ps_f", bufs=2, space="PSUM") as ps_f,
        ):
            # ---- constants ----
            ones_sq = constp.tile([P, P], dt.bfloat16, tag="ones_sq")
            nc.vector.memset(ones_sq[:], 1.0)
            ones_col = constp.tile([P, 1], dt.bfloat16, tag="ones_col")
            nc.vector.memset(ones_col[:], 1.0)

            # ---- PE pstate warmup: dummy matmuls while DMAs land ----
            nwarm = opts.get("nwarm", 14)
            if nwarm:
                warm = ps_f.tile([P, P], dt.float32, tag="psf")
                for _ in range(nwarm):
                    nc.tensor.matmul(warm[:], ones_sq[:], ones_sq[:],
                                     start=True, stop=True)

            # ---- input DMAs: features first (they gate the main loop) ----
            ft = []
            for k in range(NK):
                f_t = constp.tile([P, B], dt.bfloat16, tag=f"ft{k}")
                nc.sync.dma_start(f_t[:], featT[k * P:(k + 1) * P, :])
                ft.append(f_t)
            wgt = []
            for k in range(NK):
                w_t = constp.tile([P, B], dt.bfloat16, tag=f"wgt{k}")
                nc.sync.dma_start(w_t[:], wgT[k * P:(k + 1) * P, :])
                wgt.append(w_t)

            # ---- weight tiles; DMA issues are interleaved into the main
            # loop so out-DMA issues on the Sync queue are never starved ----
            wt_t = [constp.tile([P, CS], dt.bfloat16, tag=f"wt{k}",
                                name=f"wt_t{k}")
                    for k in range(NK)]
            pieces = _pieces()

            def emit_wt_dma(p):
                lo, hi = pieces[p]
                for k in range(NK):
                    nc.sync.dma_start(wt_t[k][:, lo:hi],
                                      wt[k * P:(k + 1) * P, lo:hi])

            WT_BOOT = min(5, NPC)
            for p in range(WT_BOOT):
                emit_wt_dma(p)

            # ---- feature normalization: fhatT = S * f / |f|, fp16 ----
            sqf = []
            for k in range(NK):
                s_t = fmp.tile([P, B], dt.bfloat16, tag="sqf", bufs=2)
                nc.vector.tensor_mul(s_t[:], ft[k][:], ft[k][:])
                sqf.append(s_t)
            ssf = ps_f.tile([P, B], dt.float32, tag="psf")
            for k in range(NK):
                nc.tensor.matmul(ssf[:], ones_sq[:], sqf[k][:],
                                 start=(k == 0), stop=(k == NK - 1))
            invfS = constp.tile([P, B], dt.bfloat16, tag="invfS")
            # rsqrt(ssf / S^2) = S / |f|
            nc.scalar.activation(invfS[:], ssf[:], Act.Abs_reciprocal_sqrt,
                                 bias=0.0, scale=1.0 / (S_SCALE * S_SCALE))
            fhat = []
            for k in range(NK):
                fh = constp.tile([P, B], dt.bfloat16, tag=f"fhat{k}")
                nc.vector.tensor_tensor(out=fh[:], in0=ft[k][:], in1=invfS[:],
                                        op=Alu.mult)
                fhat.append(fh)

            # ---- resident norm tiles ----
            partial = constp.tile([P, CS], dt.bfloat16, tag="partial")
            invw = constp.tile([P, NCH], dt.float32, tag="invw")

            margin_done = [False]

            sqwg = []
            fg = []

            def emit_margin_pre():
                # squared wg and fhat*wg products; DVE only, emitted early
                for k in range(NK):
                    s_t = fmp.tile([P, B], dt.bfloat16, tag="sqwg", bufs=2,
                                   name=f"sqwg{k}")
                    nc.vector.tensor_mul(s_t[:], wgt[k][:], wgt[k][:])
                    sqwg.append(s_t)
                for k in range(NK):
                    g_t = fmp.tile([P, B], dt.bfloat16, tag="fg", bufs=2,
                                   name=f"fg{k}")
                    nc.vector.tensor_mul(g_t[:], fhat[k][:], wgt[k][:])
                    fg.append(g_t)

            def emit_margin():
                """ArcFace margin reductions + trig; inputs already in SBUF
                so the tensor-queue matmuls never stall."""
                margin_done[0] = True
                sswg = ps_f.tile([P, B], dt.float32, tag="psf")
                for k in range(NK):
                    nc.tensor.matmul(sswg[:], ones_sq[:], sqwg[k][:],
                                     start=(k == 0), stop=(k == NK - 1))
                # rsqrt(sswg * S^2) = 1 / (S * |wg|)
                invwg = fmp.tile([1, B], dt.bfloat16, tag="invwg", bufs=1)
                nc.scalar.activation(invwg[:], sswg[0:1, :],
                                     Act.Abs_reciprocal_sqrt,
                                     bias=0.0, scale=S_SCALE * S_SCALE)
                dot = ps_f.tile([P, B], dt.float32, tag="psf")
                for k in range(NK):
                    nc.tensor.matmul(dot[:], ones_sq[:], fg[k][:],
                                     start=(k == 0), stop=(k == NK - 1))
                # t = cos(theta); the reductions are partition-replicated,
                # so the scalar chain runs on single-partition [1, B] slices
                t = fmp.tile([1, B], dt.bfloat16, tag="t", bufs=1)
                nc.vector.tensor_tensor(out=t[:], in0=dot[0:1, :],
                                        in1=invwg[:], op=Alu.mult)
                nc.vector.tensor_scalar_min(t[:], t[:], 1.0)
                nc.vector.tensor_scalar_max(t[:], t[:], -1.0)
                om = fmp.tile([1, B], dt.bfloat16, tag="om", bufs=1)
                nc.vector.tensor_mul(om[:], t[:], t[:])
                nc.vector.tensor_scalar(out=om[:], in0=om[:], scalar1=-1.0,
                                        scalar2=1.0, op0=Alu.mult, op1=Alu.add)
                rs = fmp.tile([1, B], dt.bfloat16, tag="rs", bufs=1)
                nc.scalar.activation(rs[:], om[:], Act.Abs_reciprocal_sqrt,
                                     bias=0.0, scale=1.0)
                r = fmp.tile([1, B], dt.bfloat16, tag="r", bufs=1)
                nc.vector.tensor_mul(r[:], om[:], rs[:])   # sqrt(1 - t^2)
                a1 = fmp.tile([1, B], dt.bfloat16, tag="a1", bufs=1)
                nc.vector.tensor_scalar_mul(a1[:], t[:], math.cos(M_MARGIN))
                a2 = fmp.tile([1, B], dt.bfloat16, tag="a2", bufs=1)
                nc.vector.tensor_scalar_mul(a2[:], r[:], math.sin(M_MARGIN))
                adjA = fmp.tile([1, B], dt.bfloat16, tag="adjA", bufs=1)
                nc.vector.tensor_tensor(out=adjA[:], in0=a1[:], in1=a2[:],
                                        op=Alu.subtract)
                mask = fmp.tile([1, B], dt.int8, tag="mask", bufs=1)
                nc.vector.tensor_scalar(out=mask[:], in0=t[:], scalar1=TH,
                                        scalar2=None, op0=Alu.is_gt)
                adj = fmp.tile([1, B], dt.bfloat16, tag="adj", bufs=1)
                nc.vector.tensor_scalar_sub(adj[:], t[:], MM_)
                nc.vector.copy_predicated(adj[:], mask[:], adjA[:])
                val = fmp.tile([1, B], dt.float32, tag="val", bufs=1)
                nc.vector.tensor_scalar_mul(val[:], adj[:], S_SCALE)
                nc.sync.dma_start(vout[:], val[0:1, :])

            # ---- main pipeline over class pieces, chain runs 2 ahead ----
            chunks_of = []
            gfirst = 0
            for (lo, hi) in pieces:
                glast = min(NCH, (hi + P - 1) // P)
                chunks_of.append([(g, g * P, min(P, CS - g * P))
                                  for g in range(gfirst, glast)])
                gfirst = glast

            def emit_chain(p):
                # squares + partial sums, split across DVE / Act / GpSimd
                lo, hi = pieces[p]
                w = hi - lo
                sqa = sqp.tile([P, PW], dt.bfloat16, tag="sqa", bufs=3)
                sqb = sqp.tile([P, PW], dt.bfloat16, tag="sqb", bufs=3)
                sqc = sqp.tile([P, PW], dt.bfloat16, tag="sqc", bufs=3)
                sqd = sqp.tile([P, PW], dt.bfloat16, tag="sqd", bufs=3)
                ab = sqp.tile([P, PW], dt.bfloat16, tag="ab", bufs=3)
                cd = sqp.tile([P, PW], dt.bfloat16, tag="cd", bufs=3)
                nc.vector.tensor_mul(sqa[:, :w], wt_t[0][:, lo:hi],
                                     wt_t[0][:, lo:hi])
                nc.vector.tensor_mul(sqb[:, :w], wt_t[1][:, lo:hi],
                                     wt_t[1][:, lo:hi])
                nc.scalar.activation(sqc[:, :w], wt_t[2][:, lo:hi], Act.Square,
                                     bias=0.0, scale=1.0)
                nc.gpsimd.tensor_mul(sqd[:, :w], wt_t[3][:, lo:hi],
                                     wt_t[3][:, lo:hi])
                nc.vector.tensor_tensor(out=ab[:, :w], in0=sqa[:, :w],
                                        in1=sqb[:, :w], op=Alu.add)
                nc.gpsimd.tensor_tensor(out=cd[:, :w], in0=sqc[:, :w],
                                        in1=sqd[:, :w], op=Alu.add)
                nc.vector.tensor_tensor(out=partial[:, lo:hi], in0=ab[:, :w],
                                        in1=cd[:, :w], op=Alu.add)

            def emit_norms(p):
                # class norms: one 1-col matmul per 128-class chunk
                chs = chunks_of[p]
                nch = len(chs)
                g0 = chs[0][0]
                nps = ps_n.tile([P, 8], dt.float32, tag="nps")
                for i, (g, c0, cw) in enumerate(chs):
                    nc.tensor.matmul(nps[0:cw, i:i + 1],
                                     partial[:, c0:c0 + cw], ones_col[:],
                                     start=True, stop=True)
                cwl = chs[-1][2]
                if cwl == P:
                    nc.scalar.activation(invw[:, g0:g0 + nch], nps[:, 0:nch],
                                         Act.Abs_reciprocal_sqrt,
                                         bias=0.0, scale=1.0)
                else:  # last chunk is 84 classes; avoid unwritten PSUM rows
                    if nch > 1:
                        nc.scalar.activation(invw[:, g0:g0 + nch - 1],
                                             nps[:, 0:nch - 1],
                                             Act.Abs_reciprocal_sqrt,
                                             bias=0.0, scale=1.0)
                    nc.scalar.activation(invw[0:cwl, g0 + nch - 1:g0 + nch],
                                         nps[0:cwl, nch - 1:nch],
                                         Act.Abs_reciprocal_sqrt,
                                         bias=0.0, scale=1.0)

            def emit_mains(p):
                # cos matmuls + per-partition inv-norm scale + paired out DMA
                chs = chunks_of[p]
                last_piece = p == NPC - 1
                i = 0
                while i < len(chs):
                    pair = (not last_piece and i + 1 < len(chs))
                    osb = outp.tile([P, 2 * B], dt.bfloat16, tag="osb")
                    for j in range(2 if pair else 1):
                        g, c0, cw = chs[i + j]
                        po = ps_o.tile([P, B], dt.float32, tag="po")
                        for k in range(NK):
                            nc.tensor.matmul(po[0:cw, :],
                                             wt_t[k][:, c0:c0 + cw], fhat[k][:],
                                             start=(k == 0), stop=(k == NK - 1))
                        osl = osb[0:cw, j * B:j * B + B]
                        if (g * dve_share) % 32 < dve_share:
                            nc.vector.tensor_scalar_mul(osl, po[0:cw, :],
                                                        invw[0:cw, g:g + 1])
                        else:
                            nc.scalar.activation(osl, po[0:cw, :], Act.Copy,
                                                 bias=0.0,
                                                 scale=invw[0:cw, g:g + 1])
                    g, c0, cw = chs[i]
                    if pair:
                        dst = out[c0:c0 + 2 * P, :].rearrange(
                            "(j p) b -> p j b", j=2, p=P)
                        src = osb[:, :].rearrange("p (j b) -> p j b", j=2)
                        nc.sync.dma_start(dst, src)
                        i += 2
                    else:
                        # tail chunks: split into col halves across queues
                        q = B // 2
                        for qq in range(2):
                            nc.sync.dma_start(
                                out[c0:c0 + cw, qq * q:(qq + 1) * q],
                                osb[0:cw, qq * q:(qq + 1) * q])
                        i += 1

            emit_chain(0)
            emit_margin_pre()
            emit_chain(1)
            emit_chain(2)
            emit_norms(0)
            for p in range(NPC):
                if p + 5 < NPC:
                    emit_wt_dma(p + 5)
                if p + 3 < NPC:
                    emit_chain(p + 3)
                emit_mains(p)
                if p + 1 < NPC:
                    emit_norms(p + 1)
                if p == 2:
                    emit_margin()

    nc.compile()
    return nc


def _get_nc(opts=None):
    key = tuple(sorted((opts or {}).items()))
    if key not in _CACHE:
        _CACHE[key] = _build_nc(opts)
    return _CACHE[key]


def _enable_trace_hook():
    import sys
    import types
    try:
        import antenv.axon_hooks  # noqa: F401
        return
    except ImportError:
        pass
    import antenv
    mod = types.ModuleType("antenv.axon_hooks")
    holder = [None]
    mod.set_axon_ntff_profile_hook = lambda h: holder.__setitem__(0, h)
    mod.get_axon_ntff_profile_hook = lambda: holder[0]
    sys.modules["antenv.axon_hooks"] = mod
    antenv.axon_hooks = mod
    try:
        from trn_agent_boot.trn_boot import _ntff_profile_via_ctypes
        mod.set_axon_ntff_profile_hook(
            _ntff_profile_via_ctypes("/opt/axon/libaxon_pjrt.so"))
    except Exception:
        pass


LAST_EXEC_NS = None
LAST_RESULTS = None
_OPTS = {}


def kernel(features, labels, weight):
    global LAST_EXEC_NS, LAST_RESULTS
    import ml_dtypes
    from concourse.bass_utils import run_bass_kernel_spmd

    features = np.asarray(features)
    weight = np.asarray(weight)
    labels = np.asarray(labels).astype(np.int64)

    trace = bool(int(os.environ.get("ARCFACE_TRACE", "0")))
    if trace:
        _enable_trace_hook()

    nc = _get_nc(_OPTS.get("opts"))

    featT_np = np.ascontiguousarray(features.T.astype(ml_dtypes.bfloat16))
    wgT_np = np.ascontiguousarray(weight[labels].T.astype(ml_dtypes.bfloat16))
    wt16 = weight.astype(ml_dtypes.bfloat16)

    in_maps = []
    for c in range(NCORES):
        c0 = c * CS
        wt_c = np.ascontiguousarray(wt16[c0:c0 + CS].T)  # [D, CS] fp16
        in_maps.append({
            "wt": wt_c,
            "featT": featT_np,
            "wgT": wgT_np,
        })

    res = run_bass_kernel_spmd(nc, in_maps, core_ids=list(range(NCORES)),
                               trace=trace)
    LAST_EXEC_NS = res.exec_time_ns
    LAST_RESULTS = res

    full = np.empty((B, C), dtype=np.float32)
    for c in range(NCORES):
        full[:, c * CS:(c + 1) * CS] = res.results[c]["out"].T
    rows = np.arange(B)
    full[rows, labels] = np.asarray(res.results[0]["vout"]).reshape(B)
    return full

